# revision 24
# baseline (speedup 1.0000x reference)
"""Distance-based attention (nn_Attention_67989332296336) on 8 TRN2 NeuronCores.

Math per batch element b (S=1024, E=H=A=256):
    d2[t,j]  = |x_t|^2 + |x_j|^2 - 2 x_t.x_j
    dist     = sqrt(max(d2,0)+eps)
    scores   = w_sim*dist + b_sim
    A        = softmax_j(scores)
    G        = A @ h
    Z        = tanh([G, h] @ W_g^T + b_g)

Sharding: batch dim B=32 split over 8 cores (4 per core), weights replicated.

Per-core kernel strategy (4 batch elements per core):
  - x/h loaded via SWDGE casting DMAs (f32 DRAM -> bf16 SBUF), transposed
    on TensorE in bf16 (1 cyc/row + FWL weight loads).
  - gram via bf16 TensorE matmuls on x^T tiles; d2 assembled in PSUM:
    -2*gram with the "-0.5*|x_j|^2" row folded in via a rank-8
    block-diagonal aug matmul, and |x_t|^2 (+margin, replacing
    max(d2,0)+eps — the constant shift cancels in the softmax) applied as
    the per-partition bias of the sqrt activation (scale=-2).
  - softmax without max-subtraction or b_sim (both cancel in the
    normalize); P = exp(w*dist) is symmetric, so P^T tiles for the PV
    matmul are just column-blocks of P (no transposes).  Row sums come
    free from the exp activation's accum_out.
  - the gate is folded into the PV matmul: Z = tanh((P@hW1)/den + h@W2'
    + bg) where hW = h @ [W1|W2]^T is computed once per batch (W halves
    transposed on-chip once per kernel, bg added via a K=1 aug matmul).
  - ScalarE table-set discipline: all Sqrt precede all Exp/Tanh via
    same-engine scheduler deps, so the activation table loads exactly
    twice; squares for |x|^2 run on VectorE.
  - a ~4us dummy-matmul burst at kernel start trips the PE HAM clock
    gate to 8/8 under the initial DMAs.
"""

import sys

import numpy as np

if "/opt/trn_rl_repo" not in sys.path:
    sys.path.append("/opt/trn_rl_repo")

import concourse.bacc as bacc
import concourse.bass as bass
import concourse.mybir as mybir
import concourse.tile as tile
from concourse.bass import ts
from concourse.bass_utils import run_bass_kernel_spmd
from concourse.masks import make_identity

F32 = mybir.dt.float32
F32R = mybir.dt.float32r
BF16 = mybir.dt.bfloat16
AF = mybir.ActivationFunctionType
OP = mybir.AluOpType

S = 1024
B = 32
NCORES = 8
BS = B // NCORES  # batches per core
E = 256
H = 256
A = 256
NT = S // 128  # 8 t-tiles
MARGIN = 4.0  # replaces max(d2,0)+eps; absorbs f32r rounding (cancels in softmax)


def _r(ap):
    return ap.bitcast(F32R)


def build_graph():
    nc = bacc.Bacc("TRN2", target_bir_lowering=False, debug=False)

    x_ext = nc.declare_dram_parameter("x", [S, BS, E], F32, isOutput=False)
    h_ext = nc.declare_dram_parameter("h", [S, BS, H], F32, isOutput=False)
    w_ext = nc.declare_dram_parameter("w_sim", [1, 1], F32, isOutput=False)
    wg_ext = nc.declare_dram_parameter("W_g", [A, 2 * H], F32, isOutput=False)
    bg_ext = nc.declare_dram_parameter("b_g", [1, A], F32, isOutput=False)
    out_ext = nc.declare_dram_parameter("out", [S, BS, A], F32, isOutput=True)

    with tile.TileContext(nc) as tc:
        with (
            tc.tile_pool(name="consts", bufs=1) as consts,
            tc.tile_pool(name="dist", bufs=BS) as distp,
            tc.tile_pool(name="work", bufs=2) as work,
            tc.tile_pool(name="nat", bufs=4) as natp,
            tc.tile_pool(name="small", bufs=2) as smallp,
            tc.tile_pool(name="zt", bufs=3) as ztp,
            tc.tile_pool(name="ps_bigb", bufs=2, space="PSUM") as psbb,
            tc.tile_pool(name="ps_f32", bufs=6, space="PSUM") as psf,
        ):
            # PE HAM warm-up: depends only on one fast DVE memset, so the
            # dense matmul burst starts ~immediately and trips the clock
            # gate to 8/8 while the input DMAs are still in flight.
            warm_in = consts.tile([128, 128], BF16)
            nc.vector.memset(warm_in, 1.0)
            warm_ps = psf.tile([128, 512], F32, tag="big")
            for _ in range(44):
                nc.tensor.matmul(
                    warm_ps[:, 0:128], warm_in[:], warm_in[:], start=True, stop=True
                )

            # prefetch all x and h batches (SWDGE casting DMAs)
            xnat_list = []
            for b in range(BS):
                xnat = natp.tile([128, NT, E], BF16, tag="xnat")
                xnat_list.append(xnat)
                nc.gpsimd.dma_start(
                    out=xnat,
                    in_=x_ext[:, b, :].rearrange("(i p) e -> p i e", p=128),
                )
            hnat_list = []
            for b in range(BS):
                hnat = natp.tile([128, NT, H], BF16, tag="hnat")
                hnat_list.append(hnat)
                nc.gpsimd.dma_start(
                    out=hnat,
                    in_=h_ext[:, b, :].rearrange("(i p) e -> p i e", p=128),
                )

            # ---------------- constants ----------------
            ident = consts.tile([128, 128], F32)
            make_identity(nc, ident)
            identb = consts.tile([128, 128], BF16)
            nc.vector.tensor_copy(identb, ident)
            ones_stage = consts.tile([8, 128], F32)
            nc.vector.memset(ones_stage, 1.0)
            ones_row = consts.tile([1, 128], BF16)
            nc.vector.tensor_copy(ones_row, ones_stage[0:1, :])
            ones8 = consts.tile([8, 128], BF16)
            nc.vector.tensor_copy(ones8, ones_stage)
            zero_stage = consts.tile([8, S], F32)
            nc.vector.memset(zero_stage, 0.0)
            blockdiag = consts.tile([8, S], BF16)
            nc.vector.tensor_copy(blockdiag, zero_stage)

            w_col = consts.tile([128, 1], F32)
            nc.sync.dma_start(out=w_col, in_=w_ext[:].partition_broadcast(128))
            bg_stage = consts.tile([1, A], F32)
            nc.sync.dma_start(out=bg_stage, in_=bg_ext[:])
            bg_row = consts.tile([1, A], BF16)
            nc.vector.tensor_copy(bg_row, bg_stage)

            # W_g (A, 2H) -> W12T: 2 k-tiles of (128hd, [A | A]) used as hW rhs
            wnat = consts.tile([128, 2, 2 * H], F32)
            nc.sync.dma_start(
                out=wnat, in_=wg_ext[:].rearrange("(m p) k -> p m k", m=2)
            )
            w12t = consts.tile([128, 2, 2 * H], BF16)
            for k2 in range(2):
                ps = psf.tile([128, 512], F32, tag="big")
                for w in range(2):
                    for m in range(2):
                        nc.tensor.transpose(
                            ps[:, w * 256 + m * 128 : w * 256 + (m + 1) * 128],
                            wnat[:, m, w * 256 + k2 * 128 : w * 256 + (k2 + 1) * 128],
                            ident[:],
                        )
                nc.vector.tensor_copy(w12t[:, k2, :], ps[:])

            # ---------------- phase 1: distances ----------------
            d_tiles = []
            sqrt_instrs = []
            for b in range(BS):
                xT = work.tile([128, 2, S], BF16, tag="xT")
                sqmcol = smallp.tile([128, NT], F32, tag="sqm")
                biasp = smallp.tile([128, NT], F32, tag="bias")
                d_b = distp.tile([128, NT, S], BF16, tag="D")
                d_tiles.append(d_b)

                xnat = xnat_list[b]

                # transpose pairs of x tiles: psum [T0e0|T0e1|T1e0|T1e1]
                for p2 in range(NT // 2):
                    i0, i1 = 2 * p2, 2 * p2 + 1
                    ps = psbb.tile([128, 512], BF16, tag="bigb")
                    for t2, i in enumerate((i0, i1)):
                        for k2 in range(2):
                            nc.tensor.transpose(
                                ps[:, t2 * 256 + k2 * 128 : t2 * 256 + (k2 + 1) * 128],
                                xnat[:, i, ts(k2, 128)],
                                identb[:],
                            )
                    # dst (k2, t2, f) ; src (t2, k2, f)
                    nc.vector.tensor_copy(
                        xT[:, :, i0 * 128 : i0 * 128 + 256].rearrange(
                            "p k (t f) -> p t k f", t=2
                        ),
                        ps[:].rearrange("p (t k f) -> p t k f", t=2, k=2),
                    )

                # sqmcol[:, i] = |x_t|^2 per-partition, per t-tile (DVE)
                for i in range(NT):
                    scr = smallp.tile([128, E], F32, tag="scr")
                    nc.vector.scalar_tensor_tensor(
                        out=scr,
                        in0=xnat[:, i, :],
                        scalar=1.0,
                        in1=xnat[:, i, :],
                        op0=OP.mult,
                        op1=OP.mult,
                        accum_out=sqmcol[:, i : i + 1],
                    )
                # bias = |x_t|^2 + MARGIN
                nc.vector.tensor_scalar_add(out=biasp, in0=sqmcol, scalar1=MARGIN)
                # block-diagonal (8, S) holding -0.5*|x_j|^2
                sqmb = smallp.tile([128, NT], BF16, tag="sqmb")
                nc.vector.tensor_copy(sqmb[:], sqmcol[:])
                sq8 = psbb.tile([8, 128], BF16, tag="bigb")
                nc.tensor.transpose(sq8[:], sqmb[:], identb[:])
                sq8sb = smallp.tile([8, 128], BF16, tag="sq8sb")
                nc.vector.tensor_scalar_mul(sq8sb[:], sq8[:], -0.5)
                # scatter row k to blockdiag[k, 128k:128(k+1)] via a strided DMA
                bd = blockdiag[:]
                diag_view = bass.AP(
                    tensor=bd.tensor, offset=bd.offset, ap=[[S + 128, NT], [1, 128]]
                )
                nc.sync.dma_start(out=diag_view, in_=sq8sb[:])

                for i in range(NT):
                    d2a = psf.tile([128, 512], F32, tag="big")
                    d2b = psf.tile([128, 512], F32, tag="big")
                    d2h = (d2a, d2b)
                    for k, lhsT in enumerate(
                        (xT[:, 0, ts(i, 128)], xT[:, 1, ts(i, 128)], ones8[:])
                    ):
                        rhs = (xT[:, 0, :], xT[:, 1, :], blockdiag)[k]
                        for hf in range(2):
                            nc.tensor.matmul(
                                d2h[hf][:],
                                lhsT,
                                rhs[:, ts(hf, 512)],
                                start=(k == 0),
                                stop=(k == 2),
                            )
                    for hf in range(2):
                        # dist = sqrt(-2*psum + |x_t|^2 + MARGIN)
                        si = nc.scalar.activation(
                            out=d_b[:, i, ts(hf, 512)],
                            in_=d2h[hf][:],
                            func=AF.Sqrt,
                            bias=biasp[:, i : i + 1],
                            scale=-2.0,
                        )
                        sqrt_instrs.append(si)


            # ---------------- phase 2: softmax + PV + gate ----------------
            for b in range(BS):
                hT = work.tile([128, 2, S], BF16, tag="hT")
                hw = work.tile([128, NT, 2 * H], BF16, tag="hw")
                p_b = work.tile([128, NT, S], BF16, tag="P")

                hnat = hnat_list[b]
                for p2 in range(NT // 2):
                    i0, i1 = 2 * p2, 2 * p2 + 1
                    ps = psbb.tile([128, 512], BF16, tag="bigb")
                    for t2, i in enumerate((i0, i1)):
                        for k2 in range(2):
                            nc.tensor.transpose(
                                ps[:, t2 * 256 + k2 * 128 : t2 * 256 + (k2 + 1) * 128],
                                hnat[:, i, ts(k2, 128)],
                                identb[:],
                            )
                    nc.vector.tensor_copy(
                        hT[:, :, i0 * 128 : i0 * 128 + 256].rearrange(
                            "p k (t f) -> p t k f", t=2
                        ),
                        ps[:].rearrange("p (t k f) -> p t k f", t=2, k=2),
                    )

                # hW = h @ [W1|W2]^T (+ bg on the W2 half)
                for m in range(NT):
                    ps = psf.tile([128, 512], F32, tag="big")
                    nc.tensor.matmul(
                        ps[:],
                        hT[:, 0, ts(m, 128)],
                        w12t[:, 0, :],
                        start=True,
                        stop=False,
                    )
                    nc.tensor.matmul(
                        ps[:],
                        hT[:, 1, ts(m, 128)],
                        w12t[:, 1, :],
                        start=False,
                        stop=False,
                    )
                    nc.tensor.matmul(
                        ps[:, 256:512],
                        ones_row[:],
                        bg_row[:],
                        start=False,
                        stop=True,
                    )
                    nc.vector.tensor_copy(hw[:, m, :], ps[:])

                den = smallp.tile([128, NT], F32, tag="den")
                rp = smallp.tile([128, NT], F32, tag="rp")
                # P = exp(w * dist); softmax denominators via accum_out
                for i in range(NT):
                    ei = nc.scalar.activation(
                        out=p_b[:, i, :],
                        in_=d_tiles[b][:, i, :],
                        func=AF.Exp,
                        scale=w_col[:, 0:1],
                        accum_out=den[:, i : i + 1],
                    )
                    for si in sqrt_instrs:
                        tile.add_dep_helper(
                            ei.ins, si.ins, sync=False, reason="act-table-order"
                        )
                nc.vector.reciprocal(rp[:], den[:])

                # PV with hw as the stationary operand: one weight load per
                # (a-half, k) serves two N=512 matmuls over the symmetric P
                # (moving).  Result is Z_pv^T (a, t); transposed back per tile.
                zsb = work.tile([128, 2, S], BF16, tag="Zsb")
                for ah in range(2):
                    pza = psf.tile([128, 512], F32, tag="big")
                    pzb = psf.tile([128, 512], F32, tag="big")
                    pzs = (pza, pzb)
                    for k in range(NT):
                        for th in range(2):
                            nc.tensor.matmul(
                                pzs[th][:],
                                hw[:, k, ts(ah, 128)],
                                p_b[:, k, ts(th, 512)],
                                start=(k == 0),
                                stop=(k == NT - 1),
                            )
                    for th in range(2):
                        nc.vector.tensor_copy(zsb[:, ah, ts(th, 512)], pzs[th][:])
                for i in range(NT):
                    pzt = psbb.tile([128, 256], BF16, tag="bigb")
                    for ah in range(2):
                        nc.tensor.transpose(
                            pzt[:, ts(ah, 128)], zsb[:, ah, ts(i, 128)], identb[:]
                        )
                    zs = ztp.tile([128, A], F32, tag="zs")
                    nc.vector.scalar_tensor_tensor(
                        out=zs,
                        in0=pzt[:],
                        scalar=rp[:, i : i + 1],
                        in1=hw[:, i, A : 2 * A],
                        op0=OP.mult,
                        op1=OP.add,
                    )
                    zo = ztp.tile([128, A], F32, tag="zo")
                    nc.scalar.activation(out=zo, in_=zs, func=AF.Tanh)
                    nc.sync.dma_start(out=out_ext[ts(i, 128), b, :], in_=zo)

    nc.compile()
    return nc


_CACHED = {}


def _get_graph():
    if "nc" not in _CACHED:
        _CACHED["nc"] = build_graph()
    return _CACHED["nc"]


def _run(inputs, trace=False, **kw):
    nc = _get_graph()
    x = np.asarray(inputs["x"], dtype=np.float32)
    h = np.asarray(inputs["h"], dtype=np.float32)
    w_sim = np.asarray(inputs["w_sim"], dtype=np.float32).reshape(1, 1)
    W_g = np.ascontiguousarray(np.asarray(inputs["W_g"], dtype=np.float32))
    b_g = np.asarray(inputs["b_g"], dtype=np.float32).reshape(1, A)
    in_maps = []
    for c in range(NCORES):
        in_maps.append(
            {
                "x": np.ascontiguousarray(x[:, c * BS : (c + 1) * BS, :]),
                "h": np.ascontiguousarray(h[:, c * BS : (c + 1) * BS, :]),
                "w_sim": w_sim,
                "W_g": W_g,
                "b_g": b_g,
            }
        )
    res = run_bass_kernel_spmd(nc, in_maps, list(range(NCORES)), trace=trace, **kw)
    out = np.concatenate([res.results[c]["out"] for c in range(NCORES)], axis=1)
    return out, res


def kernel(**inputs):
    out, _ = _run(inputs, trace=False)
    return out


if __name__ == "__main__":
    rng = np.random.default_rng(0)
    ins = {
        "x": rng.standard_normal((S, B, E), dtype=np.float32),
        "h": rng.standard_normal((S, B, H), dtype=np.float32),
        "w_sim": np.array([0.03], dtype=np.float32),
        "b_sim": np.array([0.01], dtype=np.float32),
        "W_g": (rng.standard_normal((A, 2 * H)) * 0.05).astype(np.float32),
        "b_g": np.zeros(A, dtype=np.float32),
    }
    out = kernel(**ins)
    print("out", out.shape, out.dtype, np.abs(out).mean())


# revision 25
# speedup vs baseline: 1.3114x; 1.3114x over previous
"""Distance-based attention (nn_Attention_67989332296336) on 8 TRN2 NeuronCores.

Math per batch element b (S=1024, E=H=A=256):
    d2[t,j]  = |x_t|^2 + |x_j|^2 - 2 x_t.x_j
    dist     = sqrt(max(d2,0)+eps)
    scores   = w_sim*dist + b_sim
    A        = softmax_j(scores)
    G        = A @ h
    Z        = tanh([G, h] @ W_g^T + b_g)

Sharding: batch dim B=32 split over 8 cores (4 per core), weights replicated.

Per-core kernel strategy (4 batch elements per core):
  - x/h loaded via SWDGE casting DMAs (f32 DRAM -> bf16 SBUF), transposed
    on TensorE in bf16 (1 cyc/row + FWL weight loads).
  - gram via bf16 TensorE matmuls on x^T tiles; d2 assembled in PSUM:
    -2*gram with the "-0.5*|x_j|^2" row folded in via a rank-8
    block-diagonal aug matmul, and |x_t|^2 (+margin, replacing
    max(d2,0)+eps — the constant shift cancels in the softmax) applied as
    the per-partition bias of the sqrt activation (scale=-2).
  - softmax without max-subtraction or b_sim (both cancel in the
    normalize); P = exp(w*dist) is symmetric, so P^T tiles for the PV
    matmul are just column-blocks of P (no transposes).  Row sums come
    free from the exp activation's accum_out.
  - the gate is folded into the PV matmul: Z = tanh((P@hW1)/den + h@W2'
    + bg) where hW = h @ [W1|W2]^T is computed once per batch (W halves
    transposed on-chip once per kernel, bg added via a K=1 aug matmul).
  - ScalarE table-set discipline: all Sqrt precede all Exp/Tanh via
    same-engine scheduler deps, so the activation table loads exactly
    twice; squares for |x|^2 run on VectorE.
  - a ~4us dummy-matmul burst at kernel start trips the PE HAM clock
    gate to 8/8 under the initial DMAs.
"""

import sys

import numpy as np

if "/opt/trn_rl_repo" not in sys.path:
    sys.path.append("/opt/trn_rl_repo")

import concourse.bacc as bacc
import concourse.bass as bass
import concourse.mybir as mybir
import concourse.tile as tile
from concourse.bass import ts
from concourse.bass_utils import run_bass_kernel_spmd
from concourse.masks import make_identity

F32 = mybir.dt.float32
F32R = mybir.dt.float32r
BF16 = mybir.dt.bfloat16
AF = mybir.ActivationFunctionType
OP = mybir.AluOpType

S = 1024
B = 32
NCORES = 8
BS = B // NCORES  # batches per core
E = 256
H = 256
A = 256
NT = S // 128  # 8 t-tiles
MARGIN = 4.0  # replaces max(d2,0)+eps; absorbs f32r rounding (cancels in softmax)


def _r(ap):
    return ap.bitcast(F32R)


def build_graph():
    nc = bacc.Bacc("TRN2", target_bir_lowering=False, debug=False)

    x_ext = nc.declare_dram_parameter("x", [S, BS, E], F32, isOutput=False)
    h_ext = nc.declare_dram_parameter("h", [S, BS, H], F32, isOutput=False)
    w_ext = nc.declare_dram_parameter("w_sim", [1, 1], F32, isOutput=False)
    wg_ext = nc.declare_dram_parameter("W_g", [A, 2 * H], F32, isOutput=False)
    bg_ext = nc.declare_dram_parameter("b_g", [1, A], F32, isOutput=False)
    out_ext = nc.declare_dram_parameter("out", [S, BS, A], F32, isOutput=True)

    with tile.TileContext(nc) as tc:
        with (
            tc.tile_pool(name="consts", bufs=1) as consts,
            tc.tile_pool(name="dist", bufs=BS) as distp,
            tc.tile_pool(name="work", bufs=2) as work,
            tc.tile_pool(name="nat", bufs=4) as natp,
            tc.tile_pool(name="small", bufs=2) as smallp,
            tc.tile_pool(name="zt", bufs=3) as ztp,
            tc.tile_pool(name="ps_bigb", bufs=2, space="PSUM") as psbb,
            tc.tile_pool(name="ps_f32", bufs=6, space="PSUM") as psf,
        ):
            # PE HAM warm-up: depends only on one fast DVE memset, so the
            # dense matmul burst starts ~immediately and trips the clock
            # gate to 8/8 while the input DMAs are still in flight.
            warm_in = consts.tile([128, 128], BF16)
            nc.vector.memset(warm_in, 1.0)
            warm_ps = psf.tile([128, 512], F32, tag="big")
            for _ in range(44):
                nc.tensor.matmul(
                    warm_ps[:, 0:128], warm_in[:], warm_in[:], start=True, stop=True
                )

            # prefetch all x and h batches (SWDGE casting DMAs)
            xnat_list = []
            for b in range(BS):
                xnat = natp.tile([128, NT, E], BF16, tag="xnat")
                xnat_list.append(xnat)
                nc.gpsimd.dma_start(
                    out=xnat,
                    in_=x_ext[:, b, :].rearrange("(i p) e -> p i e", p=128),
                )
            hnat_list = []
            for b in range(BS):
                hnat = natp.tile([128, NT, H], BF16, tag="hnat")
                hnat_list.append(hnat)
                nc.gpsimd.dma_start(
                    out=hnat,
                    in_=h_ext[:, b, :].rearrange("(i p) e -> p i e", p=128),
                )

            # ---------------- constants ----------------
            ident = consts.tile([128, 128], F32)
            make_identity(nc, ident)
            identb = consts.tile([128, 128], BF16)
            nc.vector.tensor_copy(identb, ident)
            ones_stage = consts.tile([8, 128], F32)
            nc.vector.memset(ones_stage, 1.0)
            ones_row = consts.tile([1, 128], BF16)
            nc.vector.tensor_copy(ones_row, ones_stage[0:1, :])
            ones8 = consts.tile([8, 128], BF16)
            nc.vector.tensor_copy(ones8, ones_stage)
            zero_stage = consts.tile([8, S], F32)
            nc.vector.memset(zero_stage, 0.0)
            blockdiag = consts.tile([8, S], BF16)
            nc.vector.tensor_copy(blockdiag, zero_stage)

            w_col = consts.tile([128, 1], F32)
            nc.sync.dma_start(out=w_col, in_=w_ext[:].partition_broadcast(128))
            bg_stage = consts.tile([1, A], F32)
            nc.sync.dma_start(out=bg_stage, in_=bg_ext[:])
            bg_row = consts.tile([1, A], BF16)
            nc.vector.tensor_copy(bg_row, bg_stage)

            # W_g (A, 2H) -> W12T: 2 k-tiles of (128hd, [A | A]) used as hW rhs
            wnat = consts.tile([128, 2, 2 * H], F32)
            nc.sync.dma_start(
                out=wnat, in_=wg_ext[:].rearrange("(m p) k -> p m k", m=2)
            )
            w12t = consts.tile([128, 2, 2 * H], BF16)
            for k2 in range(2):
                ps = psf.tile([128, 512], F32, tag="big")
                for w in range(2):
                    for m in range(2):
                        nc.tensor.transpose(
                            ps[:, w * 256 + m * 128 : w * 256 + (m + 1) * 128],
                            wnat[:, m, w * 256 + k2 * 128 : w * 256 + (k2 + 1) * 128],
                            ident[:],
                        )
                nc.vector.tensor_copy(w12t[:, k2, :], ps[:])

            # ---------------- phase 1: distances ----------------
            d_tiles = []
            sqrt_instrs = []
            for b in range(BS):
                xT = work.tile([128, 2, S], BF16, tag="xT")
                sqmcol = smallp.tile([128, NT], F32, tag="sqm")
                biasp = smallp.tile([128, NT], F32, tag="bias")
                d_b = distp.tile([128, NT, S], BF16, tag="D")
                d_tiles.append(d_b)

                xnat = xnat_list[b]

                # transpose pairs of x tiles: psum [T0e0|T0e1|T1e0|T1e1]
                for p2 in range(NT // 2):
                    i0, i1 = 2 * p2, 2 * p2 + 1
                    ps = psbb.tile([128, 512], BF16, tag="bigb")
                    for t2, i in enumerate((i0, i1)):
                        for k2 in range(2):
                            nc.tensor.transpose(
                                ps[:, t2 * 256 + k2 * 128 : t2 * 256 + (k2 + 1) * 128],
                                xnat[:, i, ts(k2, 128)],
                                identb[:],
                            )
                    # dst (k2, t2, f) ; src (t2, k2, f)
                    nc.vector.tensor_copy(
                        xT[:, :, i0 * 128 : i0 * 128 + 256].rearrange(
                            "p k (t f) -> p t k f", t=2
                        ),
                        ps[:].rearrange("p (t k f) -> p t k f", t=2, k=2),
                    )

                # sqmcol[:, i] = |x_t|^2 per-partition, per t-tile (DVE)
                for i in range(NT):
                    scr = smallp.tile([128, E], F32, tag="scr")
                    nc.vector.scalar_tensor_tensor(
                        out=scr,
                        in0=xnat[:, i, :],
                        scalar=1.0,
                        in1=xnat[:, i, :],
                        op0=OP.mult,
                        op1=OP.mult,
                        accum_out=sqmcol[:, i : i + 1],
                    )
                # bias = |x_t|^2 + MARGIN
                nc.vector.tensor_scalar_add(out=biasp, in0=sqmcol, scalar1=MARGIN)
                # block-diagonal (8, S) holding -0.5*|x_j|^2
                sqmb = smallp.tile([128, NT], BF16, tag="sqmb")
                nc.vector.tensor_copy(sqmb[:], sqmcol[:])
                sq8 = psbb.tile([8, 128], BF16, tag="bigb")
                nc.tensor.transpose(sq8[:], sqmb[:], identb[:])
                sq8sb = smallp.tile([8, 128], BF16, tag="sq8sb")
                nc.vector.tensor_scalar_mul(sq8sb[:], sq8[:], -0.5)
                # scatter row k to blockdiag[k, 128k:128(k+1)] via a strided DMA
                bd = blockdiag[:]
                diag_view = bass.AP(
                    tensor=bd.tensor, offset=bd.offset, ap=[[S + 128, NT], [1, 128]]
                )
                nc.sync.dma_start(out=diag_view, in_=sq8sb[:])

                for i in range(NT):
                    d2a = psf.tile([128, 512], F32, tag="big")
                    d2b = psf.tile([128, 512], F32, tag="big")
                    d2h = (d2a, d2b)
                    for k, lhsT in enumerate(
                        (xT[:, 0, ts(i, 128)], xT[:, 1, ts(i, 128)], ones8[:])
                    ):
                        rhs = (xT[:, 0, :], xT[:, 1, :], blockdiag)[k]
                        for hf in range(2):
                            nc.tensor.matmul(
                                d2h[hf][:],
                                lhsT,
                                rhs[:, ts(hf, 512)],
                                start=(k == 0),
                                stop=(k == 2),
                            )
                    for hf in range(2):
                        # dist = sqrt(-2*psum + |x_t|^2 + MARGIN)
                        si = nc.scalar.activation(
                            out=d_b[:, i, ts(hf, 512)],
                            in_=d2h[hf][:],
                            func=AF.Sqrt,
                            bias=biasp[:, i : i + 1],
                            scale=-2.0,
                        )
                        sqrt_instrs.append(si)


            # ---------------- phase 2: softmax + PV + gate ----------------
            for b in range(BS):
                hT = work.tile([128, 2, S], BF16, tag="hT")
                hw = work.tile([128, NT, 520], BF16, tag="hw")
                p_b = work.tile([128, NT, S], BF16, tag="P")

                hnat = hnat_list[b]
                for p2 in range(NT // 2):
                    i0, i1 = 2 * p2, 2 * p2 + 1
                    ps = psbb.tile([128, 512], BF16, tag="bigb")
                    for t2, i in enumerate((i0, i1)):
                        for k2 in range(2):
                            nc.tensor.transpose(
                                ps[:, t2 * 256 + k2 * 128 : t2 * 256 + (k2 + 1) * 128],
                                hnat[:, i, ts(k2, 128)],
                                identb[:],
                            )
                    nc.vector.tensor_copy(
                        hT[:, :, i0 * 128 : i0 * 128 + 256].rearrange(
                            "p k (t f) -> p t k f", t=2
                        ),
                        ps[:].rearrange("p (t k f) -> p t k f", t=2, k=2),
                    )

                # hW = h @ [W1|W2]^T (+ bg on the W2 half)
                for m in range(NT):
                    ps = psf.tile([128, 512], F32, tag="big")
                    nc.tensor.matmul(
                        ps[:],
                        hT[:, 0, ts(m, 128)],
                        w12t[:, 0, :],
                        start=True,
                        stop=False,
                    )
                    nc.tensor.matmul(
                        ps[:],
                        hT[:, 1, ts(m, 128)],
                        w12t[:, 1, :],
                        start=False,
                        stop=False,
                    )
                    nc.tensor.matmul(
                        ps[:, 256:512],
                        ones_row[:],
                        bg_row[:],
                        start=False,
                        stop=True,
                    )
                    hwm = hw[:, m, :]
                    dst = bass.AP(
                        tensor=hwm.tensor,
                        offset=hwm.offset,
                        ap=[hwm.ap[0], [257, 2], [1, 256]],
                    )
                    nc.vector.tensor_copy(
                        dst, ps[:].rearrange("p (u f) -> p u f", u=2)
                    )

                nc.vector.memset(hw[:, :, 256:257], 1.0)
                # P = exp(w * dist); denominators come from the PV ones-column
                for i2 in range(0, NT, 4):
                    ei = nc.scalar.activation(
                        out=p_b[:, i2 : i2 + 4, :],
                        in_=d_tiles[b][:, i2 : i2 + 4, :],
                        func=AF.Exp,
                        scale=w_col[:, 0:1],
                    )
                    for si in sqrt_instrs:
                        tile.add_dep_helper(
                            ei.ins, si.ins, sync=False, reason="act-table-order"
                        )

                for i2 in range(0, NT, 2):
                    zs = ztp.tile([128, 2, A], F32, tag="zs")
                    for u in range(2):
                        i = i2 + u
                        pv = psf.tile([128, 512], F32, tag="big")
                        for k in range(NT):
                            nc.tensor.matmul(
                                pv[:, 0 : A + 1],
                                p_b[:, k, ts(i, 128)],
                                hw[:, k, 0 : A + 1],
                                start=(k == 0),
                                stop=(k == NT - 1),
                            )
                        rp_i = smallp.tile([128, 1], F32, tag="rp_i")
                        nc.vector.reciprocal(rp_i[:], pv[:, A : A + 1])
                        nc.vector.scalar_tensor_tensor(
                            out=zs[:, u, :],
                            in0=pv[:, 0:A],
                            scalar=rp_i[:, 0:1],
                            in1=hw[:, i, 257 : 257 + A],
                            op0=OP.mult,
                            op1=OP.add,
                        )
                    zo = ztp.tile([128, 2, A], F32, tag="zo")
                    nc.scalar.activation(
                        out=zo[:].rearrange("p a b -> p (a b)"),
                        in_=zs[:].rearrange("p a b -> p (a b)"),
                        func=AF.Tanh,
                    )
                    nc.sync.dma_start(
                        out=out_ext[i2 * 128 : i2 * 128 + 256, b, :].rearrange(
                            "(u p) a -> p u a", p=128
                        ),
                        in_=zo,
                    )

    nc.compile()
    return nc


_CACHED = {}


def _get_graph():
    if "nc" not in _CACHED:
        _CACHED["nc"] = build_graph()
    return _CACHED["nc"]


def _run(inputs, trace=False, **kw):
    nc = _get_graph()
    x = np.asarray(inputs["x"], dtype=np.float32)
    h = np.asarray(inputs["h"], dtype=np.float32)
    w_sim = np.asarray(inputs["w_sim"], dtype=np.float32).reshape(1, 1)
    W_g = np.ascontiguousarray(np.asarray(inputs["W_g"], dtype=np.float32))
    b_g = np.asarray(inputs["b_g"], dtype=np.float32).reshape(1, A)
    in_maps = []
    for c in range(NCORES):
        in_maps.append(
            {
                "x": np.ascontiguousarray(x[:, c * BS : (c + 1) * BS, :]),
                "h": np.ascontiguousarray(h[:, c * BS : (c + 1) * BS, :]),
                "w_sim": w_sim,
                "W_g": W_g,
                "b_g": b_g,
            }
        )
    res = run_bass_kernel_spmd(nc, in_maps, list(range(NCORES)), trace=trace, **kw)
    out = np.concatenate([res.results[c]["out"] for c in range(NCORES)], axis=1)
    return out, res


def kernel(**inputs):
    out, _ = _run(inputs, trace=False)
    return out


if __name__ == "__main__":
    rng = np.random.default_rng(0)
    ins = {
        "x": rng.standard_normal((S, B, E), dtype=np.float32),
        "h": rng.standard_normal((S, B, H), dtype=np.float32),
        "w_sim": np.array([0.03], dtype=np.float32),
        "b_sim": np.array([0.01], dtype=np.float32),
        "W_g": (rng.standard_normal((A, 2 * H)) * 0.05).astype(np.float32),
        "b_g": np.zeros(A, dtype=np.float32),
    }
    out = kernel(**ins)
    print("out", out.shape, out.dtype, np.abs(out).mean())
